# revision 1
# baseline (speedup 1.0000x reference)
"""Trainium2 Bass kernel for the KGTM-style GRU message-passing GNN.

Reference math (per time step, T=3):
    agg_in  = A_in  @ nodes          (per batch)
    agg_out = A_in.T @ nodes
    zv = sigmoid(agg_in@W3wa.T + agg_out@W3wb.T + fn@W3u.T)
    rv = sigmoid(agg_in@W4wa.T + agg_out@W4wb.T + fn@W4u.T)
    hv = tanh   (agg_in@W5wa.T + agg_out@W5wb.T + (rv*fn)@W5u.T)
    fn' = fn + zv*(hv - fn)
    out_t = fn'@Wouta.T + x@Woutb.T + b_out

Mapping: pure data parallel over batch (8 cores x 256 batches, padded to 258
= 43 tiles of 6).  On-chip layout "L2" puts (batch-local, channel) on the
128-partition axis (6*20 = 120 partitions) and the node index n (512) on the
free axis.  Aggregation consumes nodes in layout "L1" [m, (b,h)] as the
matmul stationary operand so its output lands directly in L2:
    agg_L2[(b,h), n] = sum_m nodes_L1[m, (b,h)] * A~[m, n].
GRU gate matmuls use block-diagonal weights kron(I6, W.T) [120,120].  A PE
transpose converts fn' back to L1 for the next step's aggregation.  All
matmuls run as float32r (1 row/cycle, ~1.5e-4 rel err).
"""

import numpy as np

import concourse.bacc as bacc
import concourse.tile as tile
import concourse.mybir as mybir
from concourse.bass_utils import run_bass_kernel_spmd

F32 = mybir.dt.float32
F32R = mybir.dt.float32r

B, N, H, T = 2048, 512, 20, 3
NCORES = 8
BS = B // NCORES          # 256 batches per core
BPER = 6                  # batches per partition tile
TP = BPER * H             # 120 partitions per tile
NT = 43                   # tiles per core (43*6 = 258, 2 batches of zero pad)
BPAD = NT * BPER          # 258
MK = N // 128             # 4 contraction chunks of 128 along m

LAST_RESULTS = None       # stash of the most recent BassKernelResults


def _r(ap):
    return ap.bitcast(F32R)


def build_nc():
    nc = bacc.Bacc("TRN2", target_bir_lowering=False, debug=False,
                   num_devices=NCORES)

    xl1_d = nc.dram_tensor("xl1", [NT, 128, MK, TP], F32, kind="ExternalInput")
    xl2_d = nc.dram_tensor("xl2", [NT, TP, N], F32, kind="ExternalInput")
    ain_t_d = nc.dram_tensor("ain_t", [N, N], F32, kind="ExternalInput")
    ain_d = nc.dram_tensor("ain", [N, N], F32, kind="ExternalInput")
    wnames = ["wz_in", "wz_out", "wz_fn", "wr_in", "wr_out", "wr_fn",
              "wh_in", "wh_out", "wh_fn", "wo_fn", "wo_x"]
    w_d = {w: nc.dram_tensor(w, [TP, TP], F32, kind="ExternalInput")
           for w in wnames}
    bias_d = nc.dram_tensor("bias", [TP, 1], F32, kind="ExternalInput")
    ident_d = nc.dram_tensor("ident", [128, 128], F32, kind="ExternalInput")
    out_d = nc.dram_tensor("out", [T, NT, TP, N], F32, kind="ExternalOutput")

    AF = mybir.ActivationFunctionType
    with tile.TileContext(nc) as tc:
        with (
            tc.tile_pool(name="const", bufs=1) as cpool,
            tc.tile_pool(name="io", bufs=3) as iopool,
            tc.tile_pool(name="work", bufs=4) as wpool,
            tc.tile_pool(name="state", bufs=3) as spool,
            tc.tile_pool(name="psA", bufs=1, space="PSUM") as psA,
            tc.tile_pool(name="psB", bufs=1, space="PSUM") as psB,
        ):
            # ---- constants ----
            at_sb = cpool.tile([128, MK, N], F32R, name="at_sb")   # A_in.T rows
            a_sb = cpool.tile([128, MK, N], F32R, name="a_sb")     # A_in rows
            for k in range(MK):
                nc.sync.dma_start(at_sb[:, k, :], ain_t_d.ap()[128 * k:128 * (k + 1), :].bitcast(F32R))
                nc.sync.dma_start(a_sb[:, k, :], ain_d.ap()[128 * k:128 * (k + 1), :].bitcast(F32R))
            w_sb = {}
            for w in wnames:
                w_sb[w] = cpool.tile([TP, TP], F32R, name=f"{w}_sb")
                nc.sync.dma_start(w_sb[w][:], w_d[w].ap().bitcast(F32R))
            bias_sb = cpool.tile([TP, 1], F32, name="bias_sb")
            nc.sync.dma_start(bias_sb[:], bias_d.ap())
            ident = cpool.tile([128, 128], F32R, name="ident")
            nc.sync.dma_start(ident[:], ident_d.ap().bitcast(F32R))

            # ---- per-tile pipeline, emitted as a 3-deep wavefront ----
            # Wave w emits (i=w, t=0), (i=w-1, t=1), (i=w-2, t=2) so every
            # engine's FIFO interleaves three independent tile chains.
            st = [dict() for _ in range(NT)]

            def emit_step(i, t):
                if t == 0:
                    xl1_sb = iopool.tile([128, MK, TP], F32R, name="xl1_sb")
                    nc.sync.dma_start(xl1_sb[:], xl1_d.ap()[i].bitcast(F32R))
                    xl2_sb = iopool.tile([TP, N], F32R, name="xl2_sb", bufs=4)
                    nc.sync.dma_start(xl2_sb[:], xl2_d.ap()[i].bitcast(F32R))
                    st[i]["xl1"] = xl1_sb
                    st[i]["xl2"] = xl2_sb
                    st[i]["fn"] = xl2_sb      # step-0 node state is x itself
                    # skip-connection projection of x is step-invariant
                    ox_ps = psB.tile([TP, N], F32, name="ox_ps")
                    nc.tensor.matmul(ox_ps[:], w_sb["wo_x"][:], xl2_sb[:],
                                     start=True, stop=True)
                    ox_sb = wpool.tile([TP, N], F32, name="ox_sb", bufs=4)
                    nc.vector.tensor_copy(ox_sb[:], ox_ps[:])
                    st[i]["ox"] = ox_sb
                xl1_sb = st[i]["xl1"]
                xl2_sb = st[i]["xl2"]
                fn_sb = st[i]["fn"]
                fnl1_sb = st[i].get("fnl1")
                ox_sb = st[i]["ox"]
                if True:
                    # aggregation: agg = nodes_L1.T @ A~  -> L2 layout
                    agg_in_ps = psA.tile([TP, N], F32, name="agg_in_ps")
                    agg_out_ps = psA.tile([TP, N], F32, name="agg_out_ps")
                    lhs = xl1_sb if t == 0 else fnl1_sb
                    for k in range(MK):
                        nc.tensor.matmul(agg_in_ps[:], lhs[:, k, :],
                                         at_sb[:, k, :],
                                         start=(k == 0), stop=(k == MK - 1))
                    for k in range(MK):
                        nc.tensor.matmul(agg_out_ps[:], lhs[:, k, :],
                                         a_sb[:, k, :],
                                         start=(k == 0), stop=(k == MK - 1))
                    agg_in_sb = wpool.tile([TP, N], F32R, name="agg_in_sb")
                    agg_out_sb = wpool.tile([TP, N], F32R, name="agg_out_sb")
                    nc.scalar.copy(agg_in_sb[:], agg_in_ps[:])
                    nc.scalar.copy(agg_out_sb[:], agg_out_ps[:])

                    # gates: z and r share one 2-bank psum tile -> one sigmoid
                    zr_ps = psB.tile([TP, 2, N], F32, name="zr_ps")
                    nc.tensor.matmul(zr_ps[:, 0, :], w_sb["wz_in"][:], agg_in_sb[:], start=True, stop=False)
                    nc.tensor.matmul(zr_ps[:, 0, :], w_sb["wz_out"][:], agg_out_sb[:], start=False, stop=False)
                    nc.tensor.matmul(zr_ps[:, 0, :], w_sb["wz_fn"][:], fn_sb[:], start=False, stop=True)
                    nc.tensor.matmul(zr_ps[:, 1, :], w_sb["wr_in"][:], agg_in_sb[:], start=True, stop=False)
                    nc.tensor.matmul(zr_ps[:, 1, :], w_sb["wr_out"][:], agg_out_sb[:], start=False, stop=False)
                    nc.tensor.matmul(zr_ps[:, 1, :], w_sb["wr_fn"][:], fn_sb[:], start=False, stop=True)
                    zr_sb = wpool.tile([TP, 2, N], F32, name="zr_sb")
                    nc.scalar.activation(zr_sb[:], zr_ps[:], AF.Sigmoid)
                    z_sb = zr_sb[:, 0, :]
                    r_sb = zr_sb[:, 1, :]
                    rf_sb = wpool.tile([TP, N], F32R, name="rf_sb")
                    nc.vector.tensor_mul(rf_sb[:], r_sb, fn_sb[:].bitcast(F32))
                    # zf1 = (z-1)*fn, off the tanh critical path (GpSimd)
                    zf1_sb = wpool.tile([TP, N], F32, name="zf1_sb")
                    nc.vector.scalar_tensor_tensor(
                        zf1_sb[:], z_sb, 1.0, fn_sb[:].bitcast(F32),
                        op0=mybir.AluOpType.subtract, op1=mybir.AluOpType.mult)

                    h_ps = psB.tile([TP, N], F32, name="h_ps")
                    nc.tensor.matmul(h_ps[:], w_sb["wh_in"][:], agg_in_sb[:], start=True, stop=False)
                    nc.tensor.matmul(h_ps[:], w_sb["wh_out"][:], agg_out_sb[:], start=False, stop=False)
                    nc.tensor.matmul(h_ps[:], w_sb["wh_fn"][:], rf_sb[:], start=False, stop=True)
                    h_sb = wpool.tile([TP, N], F32, name="h_sb")
                    nc.scalar.activation(h_sb[:], h_ps[:], AF.Tanh)

                    # fn' = fn + z*(h - fn) = z*h - (z-1)*fn
                    zh_sb = wpool.tile([TP, N], F32, name="zh_sb")
                    nc.vector.tensor_mul(zh_sb[:], z_sb, h_sb[:])
                    fnn_sb = spool.tile([TP, N], F32R, name="fnn_sb", bufs=4)
                    nc.vector.tensor_sub(fnn_sb[:], zh_sb[:], zf1_sb[:])

                    # output projection: o = wo_fn@fn' + (hoisted x part) + bias
                    o_ps = psB.tile([TP, N], F32, name="o_ps")
                    nc.tensor.matmul(o_ps[:], w_sb["wo_fn"][:], fnn_sb[:], start=True, stop=True)
                    o_sb = iopool.tile([TP, N], F32, name="o_sb")
                    nc.vector.scalar_tensor_tensor(
                        o_sb[:], o_ps[:], bias_sb[:], ox_sb[:],
                        op0=mybir.AluOpType.add, op1=mybir.AluOpType.add)
                    nc.sync.dma_start(out_d.ap()[t, i], o_sb[:])

                    # transpose fn' into L1 for the next step's aggregation
                    if t < T - 1:
                        tp_ps = psA.tile([128, MK, TP], F32R, name="tp_ps")
                        fnl1_sb = spool.tile([128, MK, TP], F32R, name="fnl1_sb", bufs=4)
                        for k in range(MK):
                            nc.tensor.transpose(
                                tp_ps[:, k, :],
                                fnn_sb[:, 128 * k:128 * (k + 1)],
                                ident[0:TP, 0:TP])
                        nc.scalar.copy(fnl1_sb[:], tp_ps[:])
                        st[i]["fnl1"] = fnl1_sb
                    st[i]["fn"] = fnn_sb

            for w in range(NT + T - 1):
                for t in range(T):
                    i = w - t
                    if 0 <= i < NT:
                        emit_step(i, t)

    nc.compile()
    return nc


_NC_CACHE = None


def _get_nc():
    global _NC_CACHE
    if _NC_CACHE is None:
        _NC_CACHE = build_nc()
    return _NC_CACHE


def _host_prep(x, A_in, W3w, W3u, W4w, W4u, W5w, W5u, W_out, b_out):
    f32 = np.float32
    eye = np.eye(BPER, dtype=f32)

    def blk(w):
        return np.ascontiguousarray(np.kron(eye, np.asarray(w, f32).T))

    shared = {
        "ain_t": np.ascontiguousarray(np.asarray(A_in, f32).T),
        "ain": np.ascontiguousarray(np.asarray(A_in, f32)),
        "wz_in": blk(W3w[:, :H]), "wz_out": blk(W3w[:, H:]), "wz_fn": blk(W3u),
        "wr_in": blk(W4w[:, :H]), "wr_out": blk(W4w[:, H:]), "wr_fn": blk(W4u),
        "wh_in": blk(W5w[:, :H]), "wh_out": blk(W5w[:, H:]), "wh_fn": blk(W5u),
        "wo_fn": blk(W_out[:, :H]), "wo_x": blk(W_out[:, H:]),
        "bias": np.ascontiguousarray(
            np.tile(np.asarray(b_out, f32), BPER)[:, None]),
        "ident": np.eye(128, dtype=f32),
    }

    in_maps = []
    x = np.asarray(x, f32)
    for c in range(NCORES):
        xp = np.zeros((BPAD, N, H), f32)
        xp[:BS] = x[BS * c:BS * (c + 1)]
        # L1: [m, (b,h)] -> dram [NT, 128(p), MK(k), TP(j)], m = 128k+p
        l1 = xp.transpose(1, 0, 2).reshape(N, NT, TP).transpose(1, 0, 2)
        l1 = l1.reshape(NT, MK, 128, TP).transpose(0, 2, 1, 3)
        # L2: [(b,h), n] -> dram [NT, TP, N]
        l2 = xp.transpose(0, 2, 1).reshape(NT, TP, N)
        in_maps.append({"xl1": np.ascontiguousarray(l1),
                        "xl2": np.ascontiguousarray(l2), **shared})
    return in_maps


def kernel(x, A_in, W3w, W3u, W4w, W4u, W5w, W5u, W_out, b_out):
    global LAST_RESULTS
    nc = _get_nc()
    in_maps = _host_prep(x, A_in, W3w, W3u, W4w, W4u, W5w, W5u, W_out, b_out)
    res = run_bass_kernel_spmd(nc, in_maps, core_ids=list(range(NCORES)))
    LAST_RESULTS = res
    outs = []
    for c in range(NCORES):
        o = res.results[c]["out"]                      # [T, NT, TP, N]
        o = o.reshape(T, NT, BPER, H, N).transpose(0, 1, 2, 4, 3)
        outs.append(o.reshape(T, BPAD, N, H)[:, :BS])  # drop pad batches
    return np.ascontiguousarray(np.concatenate(outs, axis=1))



# revision 3
# speedup vs baseline: 1.0085x; 1.0085x over previous
"""Trainium2 Bass kernel for the KGTM-style GRU message-passing GNN (v8).

Reference math (per time step, T=3):
    agg_in  = A_in  @ nodes          (per batch)
    agg_out = A_in.T @ nodes
    zv = sigmoid(agg_in@W3wa.T + agg_out@W3wb.T + fn@W3u.T)
    rv = sigmoid(agg_in@W4wa.T + agg_out@W4wb.T + fn@W4u.T)
    hv = tanh   (agg_in@W5wa.T + agg_out@W5wb.T + (rv*fn)@W5u.T)
    fn' = fn + zv*(hv - fn)
    out_t = fn'@Wouta.T + x@Woutb.T + b_out

Mapping: data parallel over batch (8 cores x 256 batches, padded to 258 =
43 tiles of 6).  L2 layout: (batch-local, channel) = 120 partitions, node
index n = 512 free.  L1 layout: m on partitions (4x128), (b,h) free.

Engine/dtype plan (cost-model driven):
  - Aggregation: fp8e4m3 DoubleRow, stationary = fp8 state in L1, moving =
    premixed A-const [m, (in|out), n]*64; both directions in one psum
    [120, 2, 512] via 2 passes.
  - Gate av-parts: fp8 DoubleRow on av8 (fp8 copy of the agg psum).
  - Gate fn-parts: plain bf16 matmuls on the natural bf16 state.
  - L2->L1 transpose as matmul vs bf16 identity; psum->sbuf copy = fp8 cast.
  - W_out x-part + bias via a 121st all-ones row of the x tile, every step.
  - One merged input DMA per tile (fp8 L1 bytes + bf16 L2 bytes).

Schedule: 7-stage software pipeline over instances (i, t), one stage per
"tick"; instances ordered in groups of 11 tiles so a tile's consecutive
steps are 11 instances apart:
    s0: agg (PE); av8 copies (DVE/Pool, emitted end-of-tick)
    s1: zr passes (PE), sigmoid (Act)
    s2: rf (DVE), h passes (PE)
    s3: tanh (Act)
    s4: d, zd, fn' (DVE)
    s5: transpose + o passes (PE)
    s6: fnl1 copy (Pool), o-copy (Pool) + output DMA
Per-tick emission order is tuned so each engine's in-order FIFO sees ops
whose deps resolved in earlier ticks.  PSUM: av(2)+zr(2)+h/o tag(2)+tp(2)
= 8 banks.
"""

import numpy as np
import ml_dtypes

import concourse.bacc as bacc
import concourse.tile as tile
import concourse.mybir as mybir
from concourse.bass_utils import run_bass_kernel_spmd

F32 = mybir.dt.float32
BF16 = mybir.dt.bfloat16
FP8 = mybir.dt.float8e4
AF = mybir.ActivationFunctionType
DR = mybir.MatmulPerfMode.DoubleRow

NP_BF16 = ml_dtypes.bfloat16
NP_FP8 = ml_dtypes.float8_e4m3

B, N, H, T = 2048, 512, 20, 3
NCORES = 8
BS = B // NCORES          # 256 batches per core
BPER = 6                  # batches per partition tile
TP = BPER * H             # 120 partitions per tile
NT = 43                   # tiles per core (43*6 = 258, 2 batches of pad)
BPAD = NT * BPER          # 258
MK = N // 128             # 4 contraction chunks of 128 along m

XIN_W = MK * TP + 2 * N   # 480 fp8 bytes (L1 x) + 1024 bytes (bf16 L2 x)

SA = 64.0                 # fp8 scale on the A-mix constant
SW = 16.0                 # fp8 scale on gate av-weights
SFN = SA * SW             # = 1024: bf16 scale on gate fn-weights
SIG_SCALE = 1.0 / SFN

GRP = 9                  # tile-group width of the instance ordering
LEAD = 4                  # DMA prefetch lead, in ticks

LAST_RESULTS = None


def build_nc():
    nc = bacc.Bacc("TRN2", target_bir_lowering=False, debug=False,
                   num_devices=NCORES)

    xl1_d = nc.dram_tensor("xl1", [NT, 128, MK, 128], FP8,
                           kind="ExternalInput")
    xl2_d = nc.dram_tensor("xl2", [NT, TP + 1, N], BF16,
                           kind="ExternalInput")
    acat_d = nc.dram_tensor("acat", [128, 2, 2, 2, N], FP8,
                            kind="ExternalInput")
    wav_names = ["wz_av", "wr_av", "wh_av"]
    wav_d = {w: nc.dram_tensor(w, [128, 2, 128], FP8, kind="ExternalInput")
             for w in wav_names}
    wfn_names = ["wz_fn", "wr_fn", "wh_rf", "wo_fn"]
    wfn_d = {w: nc.dram_tensor(w, [TP, TP], BF16, kind="ExternalInput")
             for w in wfn_names}
    woxb_d = nc.dram_tensor("wo_xb", [TP + 1, TP], BF16, kind="ExternalInput")
    ident_d = nc.dram_tensor("ident", [TP, 128], BF16, kind="ExternalInput")
    out_d = nc.dram_tensor("out", [T, NT, TP, N], BF16, kind="ExternalOutput")

    with tile.TileContext(nc) as tc:
        with (
            tc.tile_pool(name="const", bufs=1) as cpool,
            tc.tile_pool(name="io", bufs=4) as iopool,
            tc.tile_pool(name="work", bufs=6) as wpool,
            tc.tile_pool(name="state", bufs=6) as spool,
            tc.tile_pool(name="ps", bufs=1, space="PSUM") as psp,
        ):
            # ---- constants ----
            acat_sb = cpool.tile([128, 2, 2, 2, N], FP8, name="acat_sb")
            nc.sync.dma_start(acat_sb[:], acat_d.ap())
            wav_sb = {}
            for w in wav_names:
                wav_sb[w] = cpool.tile([128, 2, 128], FP8, name=f"{w}_sb")
                nc.sync.dma_start(wav_sb[w][:], wav_d[w].ap())
            wfn_sb = {}
            for w in wfn_names:
                wfn_sb[w] = cpool.tile([TP, TP], BF16, name=f"{w}_sb")
                nc.sync.dma_start(wfn_sb[w][:], wfn_d[w].ap())
            woxb_sb = cpool.tile([TP + 1, TP], BF16, name="woxb_sb")
            nc.sync.dma_start(woxb_sb[:], woxb_d.ap())
            ident = cpool.tile([TP, 128], BF16, name="ident")
            nc.sync.dma_start(ident[:], ident_d.ap())

            st = [dict() for _ in range(NT)]

            def prefetch(i):
                xl1_sb = iopool.tile([128, MK, 128], FP8, name="xl1_sb",
                                     tag="xl1", bufs=2 * GRP + LEAD + 4)
                nc.sync.dma_start(xl1_sb[:], xl1_d.ap()[i])
                xl2_sb = iopool.tile([TP + 1, N], BF16, name="xl2_sb",
                                     tag="xl2", bufs=2 * GRP + LEAD + 4)
                nc.sync.dma_start(xl2_sb[:], xl2_d.ap()[i])
                st[i]["x"] = xl2_sb[:]
                st[i]["fn"] = xl2_sb[0:TP, :]
                st[i]["fnl1"] = xl1_sb[:]

            def s0_agg(i, t):
                av_psL = psp.tile([128, 2, N // 2], F32, name="av_psL",
                                  tag="avL", bufs=2)
                av_psR = psp.tile([128, 2, N // 2], F32, name="av_psR",
                                  tag="avR", bufs=2)
                st[i]["av_ps"] = (av_psL, av_psR)
                fnl1 = st[i]["fnl1"]
                for h, ps in ((0, av_psL), (1, av_psR)):
                    for j in range(2):
                        nc.tensor.matmul(
                            ps[:], fnl1[:, 2 * j:2 * j + 2, :],
                            acat_sb[:, j, :, h, :],
                            start=(j == 0), stop=(j == 1), perf_mode=DR)

            def s0_av8(i, t):
                av_psL, av_psR = st[i].pop("av_ps")
                av8L = wpool.tile([128, 2, N // 2], FP8, name="av8L",
                                  tag="av8L", bufs=5)
                av8R = wpool.tile([128, 2, N // 2], FP8, name="av8R",
                                  tag="av8R", bufs=5)
                nc.scalar.copy(av8L[:], av_psL[:])
                nc.vector.tensor_copy(av8R[:], av_psR[:])
                st[i]["av8"] = (av8L, av8R)

            def s1(i, t):
                fn = st[i]["fn"]
                av8L, av8R = st[i]["av8"]
                zr_ps = psp.tile([128, 2, 2, N // 2], F32, name="zr_ps",
                                 tag="zr", bufs=1)
                for g8, wname in ((0, "wz_av"), (1, "wr_av")):
                    nc.tensor.matmul(zr_ps[:, g8, 0, :], wav_sb[wname][:],
                                     av8L[:], start=True, stop=False,
                                     perf_mode=DR, skip_group_check=True)
                    nc.tensor.matmul(zr_ps[:, g8, 1, :], wav_sb[wname][:],
                                     av8R[:], start=True, stop=False,
                                     perf_mode=DR, skip_group_check=True)
                fnv = fn.rearrange("p (h n) -> p h n", h=2)
                nc.tensor.matmul(zr_ps[0:TP, 0, :, :], wfn_sb["wz_fn"][:],
                                 fnv, start=False, stop=True,
                                 skip_group_check=True)
                nc.tensor.matmul(zr_ps[0:TP, 1, :, :], wfn_sb["wr_fn"][:],
                                 fnv, start=False, stop=True,
                                 skip_group_check=True)
                zr_sb = wpool.tile([TP, 2, N], BF16, name="zr_sb",
                                   tag="zr_sb", bufs=6)
                nc.scalar.activation(
                    zr_sb[:].rearrange("p g (h n) -> p g h n", h=2),
                    zr_ps[0:TP, :, :, :], AF.Sigmoid, scale=SIG_SCALE)
                st[i]["zr"] = zr_sb

            def s2(i, t):
                fn = st[i]["fn"]
                av8L, av8R = st[i].pop("av8")
                zr_sb = st[i]["zr"]
                rf_sb = wpool.tile([TP, N], BF16, name="rf_sb", tag="rf_sb",
                                   bufs=4)
                nc.vector.tensor_mul(rf_sb[:], zr_sb[:, 1, :], fn)
                h_ps = psp.tile([128, 2, N // 2], F32, name="h_ps", tag="ho",
                                bufs=2)
                nc.tensor.matmul(h_ps[:, 0, :], wav_sb["wh_av"][:], av8L[:],
                                 start=True, stop=False, perf_mode=DR,
                                 skip_group_check=True)
                nc.tensor.matmul(h_ps[:, 1, :], wav_sb["wh_av"][:], av8R[:],
                                 start=True, stop=False, perf_mode=DR,
                                 skip_group_check=True)
                rfv = rf_sb[:].rearrange("p (h n) -> p h n", h=2)
                nc.tensor.matmul(h_ps[0:TP, :, :], wfn_sb["wh_rf"][:], rfv,
                                 start=False, stop=True,
                                 skip_group_check=True)
                st[i]["h_ps"] = h_ps

            def s3(i, t):
                h_ps = st[i].pop("h_ps")
                h_sb = wpool.tile([TP, N], BF16, name="h_sb", tag="h_sb",
                                  bufs=5)
                nc.scalar.activation(
                    h_sb[:].rearrange("p (h n) -> p h n", h=2),
                    h_ps[0:TP, :, :], AF.Tanh, scale=SIG_SCALE)
                st[i]["h"] = h_sb

            def s4(i, t):
                fn = st[i]["fn"]
                zr_sb = st[i].pop("zr")
                h_sb = st[i].pop("h")
                d_sb = wpool.tile([TP, N], BF16, name="d_sb", tag="d_sb",
                                  bufs=4)
                nc.gpsimd.tensor_sub(d_sb[:], h_sb[:], fn)
                zd_sb = wpool.tile([TP, N], BF16, name="zd_sb", tag="zd_sb",
                                   bufs=4)
                nc.gpsimd.tensor_mul(zd_sb[:], zr_sb[:, 0, :], d_sb[:])
                st[i]["zd"] = zd_sb

            def s4b(i, t):
                fn = st[i]["fn"]
                zd_sb = st[i].pop("zd")
                fnn_sb = spool.tile([TP, N], BF16, name="fnn_sb", tag="fnn",
                                    bufs=GRP + 5)
                nc.vector.tensor_add(fnn_sb[:], zd_sb[:], fn)
                st[i]["fnn"] = fnn_sb

            def s5(i, t):
                fnn_sb = st[i]["fnn"]
                xl2 = st[i]["x"]
                if t < T - 1:
                    tp_ps = psp.tile([128, MK, 128], F32, name="tp_ps",
                                     tag="avL", bufs=2)
                    for k in range(MK):
                        nc.tensor.matmul(
                            tp_ps[:, k, :],
                            fnn_sb[:, 128 * k:128 * (k + 1)],
                            ident[:], start=True, stop=True)
                    st[i]["tp_ps"] = tp_ps
                o_ps = psp.tile([TP, N], F32, name="o_ps", tag="ho", bufs=2)
                nc.tensor.matmul(o_ps[:], wfn_sb["wo_fn"][:], fnn_sb[:],
                                 start=True, stop=False)
                nc.tensor.matmul(o_ps[:], woxb_sb[:], xl2,
                                 start=False, stop=True)
                st[i]["o_ps"] = o_ps

            def s6(i, t):
                if t < T - 1:
                    tp_ps = st[i].pop("tp_ps")
                    fnl1n = spool.tile([128, MK, 128], FP8, name="fnl1n",
                                       tag="fnl1n", bufs=GRP + 5)
                    nc.vector.tensor_copy(fnl1n[:], tp_ps[:])
                    st[i]["fnl1"] = fnl1n[:]
                o_ps = st[i].pop("o_ps")
                o_sb = iopool.tile([TP, N], BF16, name="o_sb", tag="o_sb",
                                   bufs=4)
                nc.vector.tensor_copy(o_sb[:], o_ps[:])
                nc.sync.dma_start(out_d.ap()[t, i], o_sb[:])
                if t == T - 1:
                    st[i].clear()
                else:
                    st[i]["fn"] = st[i].pop("fnn")[:]

            # instance sequence: groups of GRP tiles, all T steps per group
            seq = []
            for g0 in range(0, NT, GRP):
                for t in range(T):
                    for i in range(g0, min(g0 + GRP, NT)):
                        seq.append((i, t))
            NSEQ = len(seq)

            def emit(j, fn_stage):
                if 0 <= j < NSEQ:
                    fn_stage(*seq[j])

            for g in range(-LEAD, NSEQ + 8):
                jp = g + LEAD
                if 0 <= jp < NSEQ and seq[jp][1] == 0:
                    prefetch(seq[jp][0])
                emit(g - 7, s6)       # fnl1 + o copies + dma (DVE first)
                emit(g - 3, s3)       # tanh(g-3)            Act first
                emit(g - 1, s1)       # zr passes + sigmoid
                emit(g, s0_agg)       # agg(g)
                emit(g - 6, s5)       # tp + o passes
                emit(g - 5, s4b)      # fn' add (DVE)
                emit(g - 2, s2)       # rf + h passes
                emit(g - 4, s4)       # d, zd (Pool)
                emit(g, s0_av8)       # av8 copies last

    nc.compile()
    return nc


_NC_CACHE = None


def _get_nc():
    global _NC_CACHE
    if _NC_CACHE is None:
        _NC_CACHE = build_nc()
    return _NC_CACHE


def _kron_T(w, scale=1.0):
    eye = np.eye(BPER, dtype=np.float32)
    return np.ascontiguousarray(
        np.kron(eye, np.asarray(w, np.float32).T * scale))


def _host_prep(x, A_in, W3w, W3u, W4w, W4u, W5w, W5u, W_out, b_out):
    f32 = np.float32
    A = np.asarray(A_in, f32)

    # acat[p, j, i, half, d*(N//2)+n] = A~(d)[m, half*(N//2)+n] * SA
    acat = np.empty((128, 2, 2, 2, N), f32)
    for j in range(2):
        for i in range(2):
            m0 = 128 * (2 * j + i)
            blk = slice(m0, m0 + 128)
            for h in range(2):
                ns = slice(h * (N // 2), (h + 1) * (N // 2))
                acat[:, j, i, h, 0:N // 2] = A.T[blk, ns] * SA
                acat[:, j, i, h, N // 2:] = A[blk, ns] * SA

    def wav(win, wout):
        out = np.zeros((128, 2, 128), f32)
        out[0:TP, 0, 0:TP] = _kron_T(win, SW)
        out[0:TP, 1, 0:TP] = _kron_T(wout, SW)
        return out

    W3w = np.asarray(W3w, f32)
    W4w = np.asarray(W4w, f32)
    W5w = np.asarray(W5w, f32)
    W_out_ = np.asarray(W_out, f32)

    woxb = np.zeros((TP + 1, TP), f32)
    woxb[0:TP, :] = _kron_T(W_out_[:, H:])
    woxb[TP, :] = np.tile(np.asarray(b_out, f32), BPER)

    shared = {
        "acat": acat.astype(NP_FP8),
        "wz_av": wav(W3w[:, :H], W3w[:, H:]).astype(NP_FP8),
        "wr_av": wav(W4w[:, :H], W4w[:, H:]).astype(NP_FP8),
        "wh_av": wav(W5w[:, :H], W5w[:, H:]).astype(NP_FP8),
        "wz_fn": _kron_T(W3u, SFN).astype(NP_BF16),
        "wr_fn": _kron_T(W4u, SFN).astype(NP_BF16),
        "wh_rf": _kron_T(W5u, SFN).astype(NP_BF16),
        "wo_fn": _kron_T(W_out_[:, :H]).astype(NP_BF16),
        "wo_xb": woxb.astype(NP_BF16),
        "ident": np.eye(TP, 128, dtype=f32).astype(NP_BF16),
    }

    in_maps = []
    x = np.asarray(x, f32)
    for c in range(NCORES):
        xp = np.zeros((BPAD, N, H), f32)
        xp[:BS] = x[BS * c:BS * (c + 1)]
        l1 = xp.transpose(1, 0, 2).reshape(N, NT, TP).transpose(1, 0, 2)
        l1 = l1.reshape(NT, MK, 128, TP).transpose(0, 2, 1, 3)
        l1p = np.zeros((NT, 128, MK, 128), f32)
        l1p[:, :, :, 0:TP] = l1
        l1b = np.ascontiguousarray(l1p).astype(NP_FP8)
        l2 = np.zeros((NT, TP + 1, N), f32)
        l2[:, 0:TP, :] = xp.transpose(0, 2, 1).reshape(NT, TP, N)
        l2[:, TP, :] = 1.0
        in_maps.append({"xl1": l1b,
                        "xl2": np.ascontiguousarray(l2).astype(NP_BF16),
                        **shared})
    return in_maps


def kernel(x, A_in, W3w, W3u, W4w, W4u, W5w, W5u, W_out, b_out):
    global LAST_RESULTS
    nc = _get_nc()
    in_maps = _host_prep(x, A_in, W3w, W3u, W4w, W4u, W5w, W5u, W_out, b_out)
    res = run_bass_kernel_spmd(nc, in_maps, core_ids=list(range(NCORES)))
    LAST_RESULTS = res
    outs = []
    for c in range(NCORES):
        o = np.asarray(res.results[c]["out"]).astype(np.float32)
        o = o.reshape(T, NT, BPER, H, N).transpose(0, 1, 2, 4, 3)
        outs.append(o.reshape(T, BPAD, N, H)[:, :BS])
    return np.ascontiguousarray(np.concatenate(outs, axis=1))
